# revision 11
# baseline (speedup 1.0000x reference)
"""Sparse-weight matmul (BiologicalModule) on 8 Trainium2 NeuronCores.

Computes: out = tanh(x @ scatter_coo(kernel_vector, nonzero_ind) + bias)
  x [32, 30000] f32, 500K COO nonzeros into a [30000, 2048] weight matrix.

Strategy (units-sharded, 256 output columns per core):
  - Never materialize the dense [30000, 2048] weight matrix. In CSC view,
    out[b, c] = sum_k v[c,k] * x[r[c,k], b].
  - Host packs a padded-CSC payload with the entry-slot axis k on SBUF
    PARTITIONS: per core, per k-split, g[k_p, (c, b)] holds the x values
    each entry touches and v[k_p, c] the entry values. The bias is folded
    in as entry slot 0 (g=1, v=bias[c]); the slot axis is cut exactly at
    max_count+1 so the payload carries no dead bytes. Slots >= FP8_CUT are
    stored in fp8-e4m3 (measured rel-l2 1.2e-2 on the target inputs vs the
    2e-2 gate); the rest in fp16.
  - For a single column the multiply+reduce IS one TensorE matmul:
        psum[32b, 1] (+)= g_col[Pk, 32b]^T @ v_col[Pk, 1]
    accumulated over the k-splits in PSUM. The whole compute runs on the
    otherwise-idle PE engine; ACT applies tanh per group of column chunks
    (PSUM -> SBUF f32); DVE/GPSIMD do nothing.
  - DMA-in streams ~3.6 MB/core in a few large chunks; per-column matmuls
    are issued in chunk order so all compute hides under the DMA stream.
    Column chunks shrink toward the end to keep the post-stream tail
    short; the bulk of the outputs flushes mid-stream.
"""

import sys

import ml_dtypes
import numpy as np

_TRN_REPO = "/opt/trn_rl_repo"
if _TRN_REPO not in sys.path:
    sys.path.insert(0, _TRN_REPO)

INPUT_DIM = 30000
UNITS = 2048
BATCH = 32
N_CORES = 8
UPC = UNITS // N_CORES  # 256 columns per core
FREE = BATCH * UPC  # 8192 free elems per k-split
FP8_CUT = 192  # slots >= this index are stored fp8-e4m3
# Columns per DMA chunk, grouped: chunks in one group share a PSUM tile and
# flush through a single ACT. Shrinking tail chunks keep the post-stream
# critical path short.
CHUNK_GROUPS = [[128], [96], [24, 8]]
assert sum(sum(g) for g in CHUNK_GROUPS) == UPC
# After which group to flush the bulk of the outputs.
FLUSH_AFTER = 1
# Column boundaries for the (small) fp8 sub-stream DMAs.
FP8_DMA_COLS = [128, 128]
assert sum(FP8_DMA_COLS) == UPC

_PROGRAM_CACHE = {}


def _splits(kp):
    """[(slot_lo, slot_hi, is_fp8), ...] for column length kp."""
    sp = [(0, min(128, kp), False)]
    if kp > 128:
        sp.append((128, min(FP8_CUT, kp), False))
    if kp > FP8_CUT:
        sp.append((FP8_CUT, kp, True))
    return sp


def _build_program(kp):
    """Build + compile the SPMD bass program for exact column length kp."""
    from concourse import bacc, tile
    import concourse.mybir as mybir

    f32 = mybir.dt.float32
    f16 = mybir.dt.float16
    f8 = mybir.dt.float8e4
    splits = _splits(kp)
    nsp = len(splits)

    nc = bacc.Bacc("TRN2", target_bir_lowering=False, debug=False,
                   num_devices=N_CORES)
    g_ds = [nc.dram_tensor(f"g{si}", [hi - lo, FREE], f8 if is8 else f16,
                           kind="ExternalInput")
            for si, (lo, hi, is8) in enumerate(splits)]
    v_d = nc.dram_tensor("vals", [128, nsp * UPC], f16, kind="ExternalInput")
    out_d = nc.dram_tensor("out", [BATCH, UPC], f32, kind="ExternalOutput")

    with tile.TileContext(nc) as tc:
        with (
            tc.tile_pool(name="persist", bufs=1) as persist,
            tc.tile_pool(name="gwork", bufs=6) as gwork,
            tc.psum_pool(name="psum", bufs=3) as psum,
        ):
            v_t = persist.tile([128, nsp * UPC], f16, tag="v")
            out_sb = persist.tile([BATCH, UPC], f32, tag="o")
            # fp8 sub-stream: few big DMAs, loaded alongside the first chunks
            f8_ts = {}
            if splits[-1][2]:
                lo, hi, _ = splits[-1]
                f8_pool = []
                fo = 0
                for di, wcols in enumerate(FP8_DMA_COLS):
                    t = persist.tile([hi - lo, wcols * BATCH], f8,
                                     tag=f"gf8_{di}", name=f"gf8_{di}")
                    f8_pool.append((fo, fo + wcols, t))
                    fo += wcols
                f8_ts = {"parts": f8_pool, "loaded": 0}

            def f8_tile(c):
                for lo_c, hi_c, t in f8_ts["parts"]:
                    if lo_c <= c < hi_c:
                        return t, lo_c
                raise AssertionError(c)

            co = 0
            ci = 0
            for gi, group in enumerate(CHUNK_GROUPS):
                gw = sum(group)
                ps = psum.tile([BATCH, gw], f32, tag="ps", name=f"ps{gi}")
                go = co
                for w in group:
                    fo, fw = co * BATCH, w * BATCH
                    g_ts = []
                    for si, (lo, hi, is8) in enumerate(splits):
                        if is8:
                            g_ts.append(None)
                            continue
                        g_t = gwork.tile([hi - lo, fw], f16, tag=f"g{si}",
                                         name=f"g{ci}_{si}")
                        nc.sync.dma_start(g_t[:], g_ds[si][:, fo:fo + fw])
                        g_ts.append(g_t)
                    if ci == 0:
                        # v rides behind the first g chunk: off the
                        # stream-start critical path, in SBUF well before
                        # the first matmul.
                        nc.sync.dma_start(v_t[:], v_d[:])
                    # fp8 sub-stream DMAs interleave with the first chunks
                    while (f8_ts and f8_ts["loaded"] < len(f8_ts["parts"])
                           and f8_ts["parts"][f8_ts["loaded"]][0] <= co):
                        lo_c, hi_c, t = f8_ts["parts"][f8_ts["loaded"]]
                        nc.sync.dma_start(
                            t[:], g_ds[-1][:, lo_c * BATCH:hi_c * BATCH])
                        f8_ts["loaded"] += 1
                    # per-column matmuls; the k-splits accumulate in PSUM
                    for j in range(w):
                        c = co + j
                        pj = c - go
                        for si, (lo, hi, is8) in enumerate(splits):
                            if is8:
                                t, lo_c = f8_tile(c)
                                lhsT = t[:, BATCH * (c - lo_c):
                                         BATCH * (c - lo_c + 1)]
                            else:
                                lhsT = g_ts[si][:, BATCH * j:BATCH * (j + 1)]
                            nc.tensor.matmul(
                                ps[:, pj:pj + 1],
                                lhsT=lhsT,
                                rhs=v_t[0:hi - lo,
                                        si * UPC + c:si * UPC + c + 1],
                                start=(si == 0),
                                stop=(si == nsp - 1),
                            )
                    co += w
                    ci += 1
                nc.scalar.activation(out_sb[:, go:go + gw], ps[:],
                                     mybir.ActivationFunctionType.Tanh)
                if gi == FLUSH_AFTER:
                    flushed = co
                    nc.scalar.dma_start(out_d[:, 0:flushed],
                                        out_sb[:, 0:flushed])
            nc.sync.dma_start(out_d[:, flushed:UPC],
                              out_sb[:, flushed:UPC])
    nc.compile()
    return nc


def _prepare(x, kernel_vector, bias, nonzero_ind):
    """Host-side shard prep. Returns (kp, per-core input dicts)."""
    x = np.asarray(x, dtype=np.float32)
    v = np.asarray(kernel_vector, dtype=np.float32).ravel()
    bias = np.asarray(bias, dtype=np.float32).ravel()
    ind = np.asarray(nonzero_ind)
    r = ind[:, 0].astype(np.int64)
    c = ind[:, 1].astype(np.int64)

    # COO .set semantics: de-duplicate (row, col), keeping the last occurrence.
    flat = r * UNITS + c
    if len(np.unique(flat)) != len(flat):
        _, last_rev = np.unique(flat[::-1], return_index=True)
        keep = np.sort(len(flat) - 1 - last_rev)
        r, c, v = r[keep], c[keep], v[keep]

    xt16 = np.ascontiguousarray(x.T).astype(np.float16)  # [INPUT_DIM, BATCH]

    # Sort by column, assign each entry its slot k within its column
    # (slot 0 is reserved for the bias entry).
    order = np.argsort(c, kind="stable")
    r_s, c_s, v_s = r[order], c[order], v[order]
    counts = np.bincount(c_s, minlength=UNITS)
    kp = int(counts.max()) + 1  # +1: bias slot
    starts = np.zeros(UNITS + 1, dtype=np.int64)
    np.cumsum(counts, out=starts[1:])
    k_s = np.arange(len(c_s), dtype=np.int64) - starts[c_s] + 1

    # g_full[c, k, b]: x row for the entry at (column c, slot k); padding 0.
    g_full = np.zeros((UNITS, kp, BATCH), dtype=np.float16)
    g_full[c_s, k_s] = xt16[r_s]
    v_full = np.zeros((UNITS, kp), dtype=np.float16)
    v_full[c_s, k_s] = v_s.astype(np.float16)
    # bias as entry slot 0: value bias[c], "x vector" of ones
    g_full[:, 0] = np.float16(1.0)
    v_full[:, 0] = bias.astype(np.float16)

    g_full = g_full.reshape(N_CORES, UPC, kp, BATCH)
    v_full = v_full.reshape(N_CORES, UPC, kp)
    splits = _splits(kp)

    in_maps = []
    for d in range(N_CORES):
        m = {}
        for si, (lo, hi, is8) in enumerate(splits):
            # [UPC, p, B] -> [p, UPC, B] -> [p, UPC*B] (c-major free dim)
            blk = g_full[d, :, lo:hi, :].transpose(1, 0, 2)
            blk = np.ascontiguousarray(blk).reshape(hi - lo, UPC * BATCH)
            m[f"g{si}"] = blk.astype(ml_dtypes.float8_e4m3fn) if is8 else blk
        v_core = np.zeros((128, len(splits) * UPC), dtype=np.float16)
        for si, (lo, hi, is8) in enumerate(splits):
            v_core[:hi - lo, si * UPC:(si + 1) * UPC] = v_full[d, :, lo:hi].T
        m["vals"] = v_core
        in_maps.append(m)
    return kp, in_maps


def _unshard(res):
    out = np.concatenate(
        [np.asarray(res.results[d]["out"]).reshape(BATCH, UPC)
         for d in range(N_CORES)], axis=1)
    return np.ascontiguousarray(out).astype(np.float32)


def _run(inputs, trace=False):
    from concourse.bass_utils import run_bass_kernel_spmd

    kp, in_maps = _prepare(**inputs)
    if kp not in _PROGRAM_CACHE:
        _PROGRAM_CACHE[kp] = _build_program(kp)
    nc = _PROGRAM_CACHE[kp]
    res = None
    for attempt in range(3):
        try:
            res = run_bass_kernel_spmd(
                nc, in_maps, list(range(N_CORES)), trace=trace,
            )
            break
        except Exception:
            # Transient device faults (e.g. NRT_EXEC_UNIT_UNRECOVERABLE)
            # clear on re-execution; re-raise only if persistent.
            if attempt == 2:
                raise
    assert res is not None
    return _unshard(res), res


def kernel(**inputs):
    out, _ = _run(inputs, trace=False)
    return out


# revision 13
# speedup vs baseline: 1.0705x; 1.0705x over previous
"""Sparse-weight matmul (BiologicalModule) on 8 Trainium2 NeuronCores.

Computes: out = tanh(x @ scatter_coo(kernel_vector, nonzero_ind) + bias)
  x [32, 30000] f32, 500K COO nonzeros into a [30000, 2048] weight matrix.

Strategy (units-sharded, 256 output columns per core):
  - Never materialize the dense [30000, 2048] weight matrix. In CSC view,
    out[b, c] = sum_k v[c,k] * x[r[c,k], b].
  - Host packs a padded-CSC payload with the entry-slot axis k on SBUF
    PARTITIONS; slot 0 carries the bias (g=1, v=bias[c]) and the slot axis
    is cut exactly at max_count+1 so the payload carries no dead bytes.
    Slots are split [0,128) fp16 / [128,192) fp16 / [192,kp) fp8-e4m3
    (measured rel-l2 1.2e-2 on the target inputs vs the 2e-2 gate).
  - For a single column the multiply+reduce IS one TensorE matmul:
        psum[32b, 1] (+)= g_col[Pk, 32b]^T @ v_col[Pk, 1]
    accumulated over the three k-splits in PSUM. The whole compute runs on
    the otherwise-idle PE engine; ACT applies tanh per group of column
    chunks (PSUM -> SBUF f32); DVE/GPSIMD do nothing.
  - DMA plan: the two fp16 splits are packed into ONE dram region per
    column chunk - the 64-row split rides on partitions 0..63 (first half
    of the chunk's columns) and 64..127 (second half), so each chunk is a
    single [128, .] DMA (few DMAs => HWDGE setup stays off the critical
    path, fine chunks => PE never waits long). The small fp8 sub-stream
    ships in two side DMAs; column chunks shrink toward the end so the
    post-stream tail is short; the bulk of the outputs flushes mid-stream.
"""

import sys

import ml_dtypes
import numpy as np

_TRN_REPO = "/opt/trn_rl_repo"
if _TRN_REPO not in sys.path:
    sys.path.insert(0, _TRN_REPO)

INPUT_DIM = 30000
UNITS = 2048
BATCH = 32
N_CORES = 8
UPC = UNITS // N_CORES  # 256 columns per core
FREE = BATCH * UPC  # 8192 free elems per full-width k-split
FP8_CUT = 192  # slots >= this index are stored fp8-e4m3
HALF_LO = 128  # slots [HALF_LO, FP8_CUT) form the partition-halved split
# Columns per DMA chunk (even widths; chunk starts even), grouped: chunks
# in one group share a PSUM tile and flush through a single ACT.
CHUNK_GROUPS = [[48], [48], [48], [48], [32], [16, 8, 8]]
assert sum(sum(g) for g in CHUNK_GROUPS) == UPC
# After which group to flush the bulk of the outputs.
FLUSH_AFTER = 3
# Column boundaries for the (small) fp8 sub-stream DMAs.
FP8_DMA_COLS = [128, 128]
assert sum(FP8_DMA_COLS) == UPC

_PROGRAM_CACHE = {}


def _build_program(kp):
    """Build + compile the SPMD bass program for exact column length kp."""
    from concourse import bacc, tile
    import concourse.mybir as mybir

    assert kp > FP8_CUT, f"kp={kp} needs the generic two-split build"
    f32 = mybir.dt.float32
    f16 = mybir.dt.float16
    f8 = mybir.dt.float8e4
    p8 = kp - FP8_CUT  # fp8 rows
    nh = FP8_CUT - HALF_LO  # halved-split rows (64)
    assert nh == 64
    chunks = []  # (col_offset, width)
    co = 0
    for g in CHUNK_GROUPS:
        for w in g:
            assert w % 2 == 0 and co % 2 == 0
            chunks.append((co, w))
            co += w
    # per-partition f16 elems per column: 32 (split0) + 16 (split1 halved)
    gm_d = nc_free = None

    nc = bacc.Bacc("TRN2", target_bir_lowering=False, debug=False,
                   num_devices=N_CORES)
    gm_d = nc.dram_tensor("gm", [128, 48 * UPC], f16, kind="ExternalInput")
    g8_d = nc.dram_tensor("g8", [p8, FREE], f8, kind="ExternalInput")
    v_d = nc.dram_tensor("vals", [128, UPC + UPC // 2 + UPC], f16,
                         kind="ExternalInput")
    out_d = nc.dram_tensor("out", [BATCH, UPC], f32, kind="ExternalOutput")
    V1, V2 = UPC, UPC + UPC // 2  # v column offsets of split1 / split2

    with tile.TileContext(nc) as tc:
        with (
            tc.tile_pool(name="persist", bufs=1) as persist,
            tc.tile_pool(name="gwork", bufs=6) as gwork,
            tc.psum_pool(name="psum", bufs=3) as psum,
        ):
            v_t = persist.tile([128, UPC + UPC // 2 + UPC], f16, tag="v")
            out_sb = persist.tile([BATCH, UPC], f32, tag="o")
            f8_parts = []
            fo8 = 0
            for di, wcols in enumerate(FP8_DMA_COLS):
                t = persist.tile([p8, wcols * BATCH], f8, tag=f"g8_{di}",
                                 name=f"g8_{di}")
                f8_parts.append((fo8, fo8 + wcols, t))
                fo8 += wcols

            def f8_tile(c):
                for lo_c, hi_c, t in f8_parts:
                    if lo_c <= c < hi_c:
                        return t, lo_c
                raise AssertionError(c)

            ci = 0
            f8_loaded = 0
            for gi, group in enumerate(CHUNK_GROUPS):
                gw = sum(group)
                ps = psum.tile([BATCH, gw], f32, tag="ps", name=f"ps{gi}")
                go = chunks[ci][0]
                for w in group:
                    co = chunks[ci][0]
                    g_t = gwork.tile([128, 48 * w], f16, tag="gm",
                                     name=f"gm{ci}")
                    nc.sync.dma_start(g_t[:],
                                      gm_d[:, 48 * co:48 * (co + w)])
                    if ci == 0:
                        # v rides behind the first g chunk: off the
                        # stream-start critical path, in SBUF well before
                        # the first matmul.
                        nc.sync.dma_start(v_t[:], v_d[:])
                    # fp8 sub-stream DMAs interleave with the first chunks
                    while (f8_loaded < len(f8_parts)
                           and f8_parts[f8_loaded][0] < co + w):
                        lo_c, hi_c, t = f8_parts[f8_loaded]
                        nc.sync.dma_start(
                            t[:], g8_d[:, lo_c * BATCH:hi_c * BATCH])
                        f8_loaded += 1
                    # per-column matmuls; the k-splits accumulate in PSUM
                    for j in range(w):
                        c = co + j
                        pj = c - go
                        # split0: slots [0,128), full width
                        nc.tensor.matmul(
                            ps[:, pj:pj + 1],
                            lhsT=g_t[:, 32 * j:32 * (j + 1)],
                            rhs=v_t[0:128, c:c + 1],
                            start=True, stop=False,
                        )
                        # split1: slots [128,192) on partition halves
                        if j < w // 2:
                            plo, jj = 0, j
                        else:
                            plo, jj = 64, j - w // 2
                        vc = V1 + co // 2 + jj
                        nc.tensor.matmul(
                            ps[:, pj:pj + 1],
                            lhsT=g_t[plo:plo + nh,
                                     32 * w + 32 * jj:32 * w + 32 * (jj + 1)],
                            rhs=v_t[plo:plo + nh, vc:vc + 1],
                            start=False, stop=False,
                        )
                        # split2: slots [192,kp), fp8
                        t8, lo_c = f8_tile(c)
                        nc.tensor.matmul(
                            ps[:, pj:pj + 1],
                            lhsT=t8[:, BATCH * (c - lo_c):
                                    BATCH * (c - lo_c + 1)],
                            rhs=v_t[0:p8, V2 + c:V2 + c + 1],
                            start=False, stop=True,
                        )
                    ci += 1
                co_end = chunks[ci - 1][0] + chunks[ci - 1][1]
                nc.scalar.activation(out_sb[:, go:go + gw], ps[:],
                                     mybir.ActivationFunctionType.Tanh)
                if gi == FLUSH_AFTER:
                    flushed = co_end
                    nc.scalar.dma_start(out_d[:, 0:flushed],
                                        out_sb[:, 0:flushed])
            nc.sync.dma_start(out_d[:, flushed:UPC],
                              out_sb[:, flushed:UPC])
    nc.compile()
    return nc


def _prepare(x, kernel_vector, bias, nonzero_ind):
    """Host-side shard prep. Returns (kp, per-core input dicts)."""
    x = np.asarray(x, dtype=np.float32)
    v = np.asarray(kernel_vector, dtype=np.float32).ravel()
    bias = np.asarray(bias, dtype=np.float32).ravel()
    ind = np.asarray(nonzero_ind)
    r = ind[:, 0].astype(np.int64)
    c = ind[:, 1].astype(np.int64)

    # COO .set semantics: de-duplicate (row, col), keeping the last occurrence.
    flat = r * UNITS + c
    if len(np.unique(flat)) != len(flat):
        _, last_rev = np.unique(flat[::-1], return_index=True)
        keep = np.sort(len(flat) - 1 - last_rev)
        r, c, v = r[keep], c[keep], v[keep]

    xt16 = np.ascontiguousarray(x.T).astype(np.float16)  # [INPUT_DIM, BATCH]

    # Sort by column, assign each entry its slot k within its column
    # (slot 0 is reserved for the bias entry).
    order = np.argsort(c, kind="stable")
    r_s, c_s, v_s = r[order], c[order], v[order]
    counts = np.bincount(c_s, minlength=UNITS)
    kp = int(counts.max()) + 1  # +1: bias slot
    starts = np.zeros(UNITS + 1, dtype=np.int64)
    np.cumsum(counts, out=starts[1:])
    k_s = np.arange(len(c_s), dtype=np.int64) - starts[c_s] + 1

    # g_full[c, k, b]: x row for the entry at (column c, slot k); padding 0.
    g_full = np.zeros((UNITS, kp, BATCH), dtype=np.float16)
    g_full[c_s, k_s] = xt16[r_s]
    v_full = np.zeros((UNITS, kp), dtype=np.float16)
    v_full[c_s, k_s] = v_s.astype(np.float16)
    # bias as entry slot 0: value bias[c], "x vector" of ones
    g_full[:, 0] = np.float16(1.0)
    v_full[:, 0] = bias.astype(np.float16)

    g_full = g_full.reshape(N_CORES, UPC, kp, BATCH)
    v_full = v_full.reshape(N_CORES, UPC, kp)
    p8 = kp - FP8_CUT
    chunks = []
    co = 0
    for g in CHUNK_GROUPS:
        for w in g:
            chunks.append((co, w))
            co += w

    in_maps = []
    for d in range(N_CORES):
        # merged fp16 stream: per chunk [split0 | split1 partition-halved]
        gm = np.zeros((128, 48 * UPC), dtype=np.float16)
        for co, w in chunks:
            base = 48 * co
            # split0: [w, 128, B] -> [128, w*B]
            a = g_full[d, co:co + w, 0:128, :].transpose(1, 0, 2)
            gm[:, base:base + 32 * w] = a.reshape(128, 32 * w)
            # split1: [w, 64, B]; first half cols -> partitions 0..63
            b = g_full[d, co:co + w, HALF_LO:FP8_CUT, :].transpose(1, 0, 2)
            h = w // 2
            gm[0:64, base + 32 * w:base + 32 * w + 32 * h] = \
                b[:, 0:h].reshape(64, 32 * h)
            gm[64:128, base + 32 * w:base + 32 * w + 32 * h] = \
                b[:, h:w].reshape(64, 32 * h)
        g8 = g_full[d, :, FP8_CUT:, :].transpose(1, 0, 2)
        g8 = np.ascontiguousarray(g8).reshape(p8, UPC * BATCH)
        v_core = np.zeros((128, UPC + UPC // 2 + UPC), dtype=np.float16)
        v_core[0:128, 0:UPC] = v_full[d, :, 0:128].T
        for co, w in chunks:
            h = w // 2
            vb = UPC + co // 2
            v_core[0:64, vb:vb + h] = v_full[d, co:co + h, HALF_LO:FP8_CUT].T
            v_core[64:128, vb:vb + h] = \
                v_full[d, co + h:co + w, HALF_LO:FP8_CUT].T
        v_core[0:p8, UPC + UPC // 2:] = v_full[d, :, FP8_CUT:].T
        in_maps.append({
            "gm": gm,
            "g8": g8.astype(ml_dtypes.float8_e4m3fn),
            "vals": v_core,
        })
    return kp, in_maps


def _unshard(res):
    out = np.concatenate(
        [np.asarray(res.results[d]["out"]).reshape(BATCH, UPC)
         for d in range(N_CORES)], axis=1)
    return np.ascontiguousarray(out).astype(np.float32)


def _run(inputs, trace=False):
    from concourse.bass_utils import run_bass_kernel_spmd

    kp, in_maps = _prepare(**inputs)
    if kp not in _PROGRAM_CACHE:
        _PROGRAM_CACHE[kp] = _build_program(kp)
    nc = _PROGRAM_CACHE[kp]
    res = None
    for attempt in range(3):
        try:
            res = run_bass_kernel_spmd(
                nc, in_maps, list(range(N_CORES)), trace=trace,
            )
            break
        except Exception:
            # Transient device faults (e.g. NRT_EXEC_UNIT_UNRECOVERABLE)
            # clear on re-execution; re-raise only if persistent.
            if attempt == 2:
                raise
    assert res is not None
    return _unshard(res), res


def kernel(**inputs):
    out, _ = _run(inputs, trace=False)
    return out
